# revision 46
# baseline (speedup 1.0000x reference)
"""Multi-head attention (B=2, S=2048, D=1024, H=16) on 8 Trainium2 NeuronCores.

Sharding: batch x head-group. Core c handles batch c//4 and heads 4*(c%4)..4*(c%4)+3
(column-parallel Wq/Wk/Wv, row-parallel Wo; partial outputs summed on host).

Per-core dataflow (all in "transposed" orientation so the PE contracts naturally):
  Q^T/K^T = W^T @ X^T   (bf16 matmuls) -> DVE bias-add -> bf16 SBUF
  V^T     = Wv^T @ Xv^T -> natural V [s, hd] via X-stationary matmul (+ ones col)
  scores^T[sk,sq] = K_h @ Q_h^T  (two heads row-packed via PE tile_position)
  P^T = exp(scores^T/8) (ACT, psum->sbuf bf16) * maskT (single DVE mul, bcast)
  attn^T[hd+1,sq] = [V_h|1]^T @ P^T  (ones row accumulates softmax denominators)
  1/sums via DVE reciprocal_approx_fast; gpsimd partition-broadcast; DVE normalize
  out[sq,do] = attnT^T @ Wo interleaved into the attention loop, DMA'd as it's made.

Schedule: the PE stream is software-pipelined — each kc2 iteration issues
scores(k) then the *previous* iteration's attnV, so the PE never stalls on the
scores->exp->mask->attnV chain. V and Q1-3 projections are interleaved into the
first two head-pair iterations so exp starts ~28us in. Scalar runs exp only
(single act-table load); out-projection+DMA stream out per sq-chunk.
"""

import numpy as np
import ml_dtypes

B, S, D, H, HD = 2, 2048, 1024, 16, 64
NCORES = 8
HPC = 4          # heads per core
DH4 = HPC * HD   # 256 projection cols per core
KCP = D // 128   # 8 contraction chunks for projections
SC = S // 512    # 4 sq chunks
KCS = S // 128   # 16 sk chunks

_CACHE = {}


def _build_nc():
    from contextlib import ExitStack

    import concourse.bacc as bacc
    import concourse.tile as tile
    from concourse import mybir

    dt = mybir.dt
    AF = mybir.ActivationFunctionType

    nc = bacc.Bacc("TRN2", target_bir_lowering=False, debug=False)

    xT = [
        nc.dram_tensor(n, [128, SC, KCP, 512], dt.bfloat16, kind="ExternalInput")
        for n in ("xqT", "xkT", "xvT")
    ]
    maskT_d = nc.dram_tensor("maskT", [128, SC, KCS, 512], dt.bfloat16, kind="ExternalInput")
    w_d = [
        nc.dram_tensor(n, [128, KCP, DH4], dt.bfloat16, kind="ExternalInput")
        for n in ("wq", "wk", "wv")
    ]
    bqkv_d = nc.dram_tensor("bqkv", [128, 2, 2], dt.float32, kind="ExternalInput")
    wo_d = nc.dram_tensor("wo", [128, 2, D], dt.bfloat16, kind="ExternalInput")
    out_d = nc.dram_tensor("out", [S, D], dt.float32, kind="ExternalOutput")

    with tile.TileContext(nc) as tc, ExitStack() as ctx:
        consts = ctx.enter_context(tc.tile_pool(name="consts", bufs=1))
        wpool = ctx.enter_context(tc.tile_pool(name="wpool", bufs=1))
        persist = ctx.enter_context(tc.tile_pool(name="persist", bufs=1))
        xtpool = ctx.enter_context(tc.tile_pool(name="xtpool", bufs=5))
        xvpool = ctx.enter_context(tc.tile_pool(name="xvpool", bufs=4))
        maskpool = ctx.enter_context(tc.tile_pool(name="maskpool", bufs=2))
        ptpool = ctx.enter_context(tc.tile_pool(name="ptpool", bufs=7))
        smalls = ctx.enter_context(tc.tile_pool(name="smalls", bufs=1))
        outpool = ctx.enter_context(tc.tile_pool(name="outpool", bufs=2))
        psp = ctx.enter_context(tc.tile_pool(name="psp", bufs=2, space="PSUM"))
        pvp = ctx.enter_context(tc.tile_pool(name="pvp", bufs=2, space="PSUM"))

        # w_sb laid out t-major so each w DMA writes one contiguous block
        w_sb = wpool.tile([128, 3, KCP, DH4], dt.bfloat16, tag="w")
        bias_sb = consts.tile([128, 2, 2], dt.float32)
        wo_sb = consts.tile([128, 2, D], dt.bfloat16)

        # ---- DMA preamble ----
        # gpsimd (SWDGE) ring: bias, wv, mask0, xv0-3, wo
        nc.gpsimd.dma_start(out=bias_sb[:, :, :], in_=bqkv_d[:, :, :])

        mtiles = [None] * SC

        def mask_dma(sc, eng):
            mtiles[sc] = maskpool.tile(
                [128, KCS, 512], dt.bfloat16, tag="mk", name=f"mk{sc}"
            )
            eng.dma_start(out=mtiles[sc][:, :, :], in_=maskT_d[:, sc, :, :])

        # Pending DMAs fair-share HBM bandwidth (SDMA round-robins across
        # queues), so only the K/Q0-critical transfers may be in flight at
        # kernel start. Everything else is issued later, gated by program
        # position on the gpsimd/vector streams (a 1-element gpsimd copy of
        # the xk3 tile stalls that stream until the K bytes have landed).
        xk_t, xq_t, xv_t = [], [], []

        def x_dma(lst, t, sc, eng, pool, nm):
            tl = pool.tile([128, KCP, 512], dt.bfloat16, tag="xt" if pool is xtpool else "xv", name=f"{nm}{sc}")
            eng.dma_start(out=tl[:, :, :], in_=xT[t][:, sc, :, :])
            lst.append(tl)

        # A single HWDGE ring drains FIFO with each DMA spread across all 16
        # SDMA engines — i.e. it is a full-bandwidth priority queue. Multiple
        # active rings fair-share HBM, which starves the critical K stream.
        # So: ALL input DMAs ride the sync ring, in deadline order.
        nc.sync.dma_start(out=w_sb[:, 1, :, :], in_=w_d[1][:, :, :])

        def x_dma_halves(lst, t, sc, eng, pool, nm):
            tl = pool.tile([128, KCP, 512], dt.bfloat16, tag="xt", name=f"{nm}{sc}")
            h = KCP // 2
            eng.dma_start(out=tl[:, 0:h, :], in_=xT[t][:, sc, 0:h, :])
            eng.dma_start(out=tl[:, h:KCP, :], in_=xT[t][:, sc, h:KCP, :])
            lst.append(tl)

        for sc in range(SC):
            x_dma_halves(xk_t, 1, sc, nc.sync, xtpool, "xk")
        nc.sync.dma_start(out=w_sb[:, 0, :, :], in_=w_d[0][:, :, :])
        x_dma_halves(xq_t, 0, 0, nc.sync, xtpool, "xq")
        nc.sync.dma_start(out=w_sb[:, 2, :, :], in_=w_d[2][:, :, :])
        x_dma(xv_t, 2, 0, nc.sync, xvpool, "xv")
        mask_dma(0, nc.sync)
        x_dma(xv_t, 2, 1, nc.sync, xvpool, "xv")
        x_dma(xv_t, 2, 2, nc.sync, xvpool, "xv")

        # ---- HAM warm-up: ~4us of tiny matmuls before xk0 lands so the
        # PE clock-gate is already at 2.4GHz when the real work starts ----
        warm = consts.tile([128, 64], dt.bfloat16)
        nc.vector.memset(warm[:, :], 0.5)
        ps_warm = psp.tile([128, 64], dt.float32, tag="ps")
        for _ in range(150):
            nc.tensor.matmul(
                ps_warm[0:64, :], lhsT=warm[:, 0:64], rhs=warm[:, :],
                start=True, stop=True,
            )

        # ---- persistent SBUF tiles ----
        qt_c = [
            persist.tile([128, 2, 512], dt.bfloat16, tag=f"qt{i}", name=f"qt{i}")
            for i in range(SC)
        ]
        kt_c = [
            persist.tile([128, 2, 512], dt.bfloat16, tag=f"kt{i}", name=f"kt{i}")
            for i in range(SC)
        ]
        vaug_c = [
            persist.tile(
                [128, 4, HPC, HD + 1], dt.bfloat16, tag=f"va{i}", name=f"va{i}"
            )
            for i in range(SC)
        ]
        attnT = persist.tile([128, 2, S], dt.bfloat16, tag="attnT")
        for i in range(SC):
            nc.vector.memset(vaug_c[i][:, :, :, HD : HD + 1], 1.0)

        # PSUM->SBUF move + bias add on the DVE (tensor_scalar) so the Scalar
        # engine runs nothing but Exp (single act-table load, no thrash).
        def proj_qk(t, sc):
            xtile = (xq_t, xk_t)[t][sc]
            ps = psp.tile([128, 1024], dt.float32, tag="ps")
            for kc in range(KCP):
                for m in range(2):
                    nc.tensor.matmul(
                        ps[:, m * 512 : (m + 1) * 512],
                        lhsT=w_sb[:, t, kc, m * 128 : (m + 1) * 128],
                        rhs=xtile[:, kc, :],
                        start=(kc == 0),
                        stop=(kc == KCP - 1),
                    )
            dst = (qt_c, kt_c)[t][sc]
            for m in range(2):
                nc.vector.tensor_scalar(
                    out=dst[:, m, :],
                    in0=ps[:, m * 512 : (m + 1) * 512],
                    scalar1=bias_sb[:, t, m : m + 1],
                    scalar2=None,
                    op0=mybir.AluOpType.add,
                )

        def proj_v(sc, js=(0, 2)):
            # two sq sub-blocks share one psum tile -> half the ring churn
            for j in js:
                po_v = psp.tile([128, 2, DH4], dt.float32, tag="ps")
                for jj in range(2):
                    for kc in range(KCP):
                        nc.tensor.matmul(
                            po_v[:, jj, :],
                            lhsT=xv_t[sc][:, kc, (j + jj) * 128 : (j + jj + 1) * 128],
                            rhs=w_sb[:, 2, kc, :],
                            start=(kc == 0),
                            stop=(kc == KCP - 1),
                        )
                nc.vector.tensor_copy(
                    out=vaug_c[sc][:, j : j + 2, :, 0:HD],
                    in_=po_v[:, :, :].rearrange("p j (h d) -> p j h d", h=4),
                )

        # ---- Phase A lead-in: K (all), Q0, V0. The rest of the projections
        # are interleaved into the first attention pairs below. mask/xq1-3
        # DMAs are issued from the vector stream so they only hit HBM once
        # the K projections have consumed their inputs.
        # rest of the inputs, still in deadline order on the sync ring
        x_dma(xq_t, 0, 1, nc.sync, xtpool, "xq")
        x_dma(xv_t, 2, 3, nc.sync, xvpool, "xv")
        x_dma(xq_t, 0, 2, nc.sync, xtpool, "xq")
        x_dma(xq_t, 0, 3, nc.sync, xtpool, "xq")
        mask_dma(1, nc.sync)
        nc.sync.dma_start(out=wo_sb[:, :, :], in_=wo_d[:, :, :])

        for sc in range(SC):
            proj_qk(1, sc)
        proj_qk(0, 0)

        # ---- Phase B: attention per (sq-chunk, head-pair) ----
        pend = [None]

        # The flush is split in two stages issued one kc2 apart so the
        # ~2.5us gpsimd partition-broadcast latency never bubbles the DVE
        # FIFO between two mask-multiplies.
        def flush_stage1(pvx, pp, scp):
            # 1/sums on DVE (approx recip, ~51 ULP), broadcast on gpsimd
            sums_sb = smalls.tile(
                [1, 1024], dt.float32, tag="sums", name=f"sums{scp}_{pp}"
            )
            nc.vector.tensor_copy(out=sums_sb[0:1, :], in_=pvx[HD : HD + 1, :])
            recip_sb = smalls.tile(
                [1, 1024], dt.float32, tag="recip", name=f"recip{scp}_{pp}"
            )
            nc.vector.reciprocal_approx_fast(
                out=recip_sb[0:1, :], in_=sums_sb[0:1, :]
            )
            bcs = smalls.tile(
                [HD, 1024], dt.float32, tag="bcs", name=f"bcs{scp}_{pp}"
            )
            nc.gpsimd.partition_broadcast(bcs[:, :], recip_sb[0:1, :])
            return bcs

        def flush_stage2(bcs, pvx, pp, scp):
            # normalize+cast on DVE; bcs has been ready for a while
            for i in range(2):
                nc.vector.tensor_mul(
                    out=attnT[
                        64 * i : 64 * (i + 1), pp, scp * 512 : (scp + 1) * 512
                    ],
                    in0=pvx[0:HD, i * 512 : (i + 1) * 512],
                    in1=bcs[0:HD, i * 512 : (i + 1) * 512],
                )

        def outproj_s1(s1):
            # one 128-row block of the output projection; PSUM->SBUF copy on
            # the Scalar engine (Copy is in every act table set) to fill its
            # bubbles and keep the DVE free for mask mults.
            po = psp.tile([128, 1024], dt.float32, tag="ps")
            for c in range(2):
                for m in range(2):
                    nc.tensor.matmul(
                        po[:, m * 512 : (m + 1) * 512],
                        lhsT=attnT[:, c, s1 * 128 : (s1 + 1) * 128],
                        rhs=wo_sb[:, c, m * 512 : (m + 1) * 512],
                        start=(c == 0),
                        stop=(c == 1),
                    )
            ot = outpool.tile([128, 1024], dt.float32, tag="ot")
            if s1 % 2:
                nc.vector.tensor_copy(out=ot[:, :], in_=po[:, :])
            else:
                nc.scalar.copy(out=ot[:, :], in_=po[:, :])
            # the last two blocks ride the idle scalar HWDGE ring so the
            # final transfers overlap instead of serializing on sync
            eng = nc.scalar if s1 >= 4 * SC - 2 and s1 % 2 else nc.sync
            eng.dma_start(
                out=out_d[s1 * 128 : (s1 + 1) * 128, :], in_=ot[:, :]
            )

        def attn_v(pv, p, pt, kc2):
            for j in range(2):
                kc = 2 * kc2 + j
                for i in range(2):
                    nc.tensor.matmul(
                        pv[:, i * 512 : (i + 1) * 512],
                        lhsT=vaug_c[kc // 4][:, kc % 4, 2 * p + i, :],
                        rhs=pt[:, j, i, :],
                        start=(kc == 0),
                        stop=(kc == KCS - 1),
                    )

        # extra PE work slotted into early pairs: (sc0,p0) projects V0-3 (each
        # just before the attnV that needs it); Q_sc projects one pair ahead
        # of its sq-chunk so the load spreads across the whole first half.
        fill_work = {
            (0, 0, 0): lambda: proj_v(0),
            (0, 0, 2): lambda: proj_v(1),
            (0, 0, 4): lambda: proj_v(2),
            (0, 0, 6): lambda: proj_v(3),
            (0, 1, 0): lambda: proj_qk(0, 1),
            (1, 0, 0): lambda: proj_qk(0, 2),
            (2, 0, 0): lambda: proj_qk(0, 3),
        }

        # The attnV pipeline (depth 2) is carried ACROSS pair and sq-chunk
        # boundaries: kc2's attnV is issued after kc2+2's scores, so the PE
        # never drains waiting for the exp->mask chain. The flush of the
        # previous pair happens at kc2==2 (its last attnV was issued at
        # kc2==1); the output projection for a finished sq-chunk is spread one
        # 128-row block per kc2 over kc2=3..6.
        pipe = []
        outq = []  # pending outproj s1 blocks, issued one per kc2
        for sc in range(SC):
            if sc + 2 < SC:
                mask_dma(sc + 2, nc.sync)
            mtile = mtiles[sc]
            for p in range(2):
                pv = pvp.tile([HD + 1, 1024], dt.float32, tag="pv")
                for kc2 in range(KCS // 2):
                    pt = ptpool.tile([128, 2, 2, 512], dt.bfloat16, tag="pt")
                    for j in range(2):
                        kc = 2 * kc2 + j
                        ps = psp.tile([128, 1024], dt.float32, tag="ps")
                        nc.tensor.matmul(
                            ps[:, 0:512],
                            lhsT=kt_c[kc // 4][
                                0:64, p, (kc % 4) * 128 : (kc % 4 + 1) * 128
                            ],
                            rhs=qt_c[sc][0:64, p, :],
                            start=True,
                            stop=True,
                        )
                        nc.tensor.matmul(
                            ps[:, 512:1024],
                            lhsT=kt_c[kc // 4][
                                64:128, p, (kc % 4) * 128 : (kc % 4 + 1) * 128
                            ],
                            rhs=qt_c[sc][64:128, p, :],
                            start=True,
                            stop=True,
                            tile_position=(64, 0),
                        )
                        nc.scalar.activation(
                            out=pt[:, j, :, :].rearrange("s h q -> s (h q)"),
                            in_=ps[:, :],
                            func=AF.Exp,
                            scale=0.125,
                        )
                    # the kc2-4 attnV goes after this kc2's scores
                    if len(pipe) >= 3:
                        attn_v(*pipe.pop(0))
                    # flush checks AFTER the attn_v pop: at kc2==2 the
                    # previous pair's last attnV has just been issued, so
                    # stage1's sums read sees the complete accumulator
                    if kc2 == 2 and pend[0] is not None and len(pend[0]) == 3:
                        pend[0] = pend[0] + (flush_stage1(*pend[0]),)
                    if kc2 == 3 and pend[0] is not None and len(pend[0]) == 4:
                        pvx, pp, scp, bcs = pend[0]
                        flush_stage2(bcs, pvx, pp, scp)
                        pend[0] = None
                        if pp == 1:
                            outq.extend(range(4 * scp, 4 * scp + 4))
                    w = fill_work.pop((sc, p, kc2), None)
                    if w is not None:
                        w()
                    if kc2 >= 4 and outq and sc < SC - 1:
                        outproj_s1(outq.pop(0))
                    for j in range(2):
                        nc.vector.tensor_mul(
                            out=pt[:, j, :, :],
                            in0=pt[:, j, :, :],
                            in1=mtile[:, 2 * kc2 + j, :]
                            .rearrange("p (i q) -> p i q", i=1)
                            .broadcast_to([128, 2, 512]),
                        )
                    pipe.append((pv, p, pt, kc2))
                pend[0] = (pv, p, sc)

        while pipe:
            attn_v(*pipe.pop(0))
        if pend[0] is not None:
            pvx, pp, scp = pend[0]
            bcs = flush_stage1(pvx, pp, scp)
            # outproj(2) backlog fills the PE while the broadcast runs
            for s1 in outq:
                outproj_s1(s1)
            outq = []
            # normalize per 128-col block so each final outproj block starts
            # the moment its columns land
            for b in range(4):
                for i in range(2):
                    nc.vector.tensor_mul(
                        out=attnT[
                            64 * i : 64 * (i + 1),
                            pp,
                            scp * 512 + b * 128 : scp * 512 + (b + 1) * 128,
                        ],
                        in0=pvx[0:HD, i * 512 + b * 128 : i * 512 + (b + 1) * 128],
                        in1=bcs[0:HD, i * 512 + b * 128 : i * 512 + (b + 1) * 128],
                    )
                outproj_s1(4 * scp + b)
            pend[0] = None
        for s1 in outq:
            outproj_s1(s1)

    nc.compile()
    return nc


def _prep_inputs(query, key_, value, mask, Wq, bq, Wk, bk, Wv, bv, Wo, bo):
    bf16 = ml_dtypes.bfloat16
    f32 = np.float32

    def _xblock(x):
        # [S, D] -> X^T [D, S] -> [128p, SC, KCP, 512] (contiguous per partition)
        xt = np.asarray(x, f32).T.astype(bf16)
        return np.ascontiguousarray(
            xt.reshape(KCP, 128, SC, 512).transpose(1, 2, 0, 3)
        )

    def _mblock(mk):
        mt = np.asarray(mk).T.astype(bf16)  # maskT [sk, sq]
        return np.ascontiguousarray(
            mt.reshape(KCS, 128, SC, 512).transpose(1, 2, 0, 3)
        )

    per_batch = []
    for b in range(B):
        per_batch.append(
            {
                "xqT": _xblock(query[b]),
                "xkT": _xblock(key_[b]),
                "xvT": _xblock(value[b]),
                "maskT": _mblock(mask[b, 0]),
            }
        )
    in_maps = []
    for c in range(NCORES):
        b, hq = divmod(c, NCORES // B)
        cs = slice(DH4 * hq, DH4 * (hq + 1))
        m = dict(per_batch[b])

        def _wblock(w):
            ws = np.asarray(w, f32)[:, cs].astype(bf16)  # [D, 256]
            return np.ascontiguousarray(
                ws.reshape(KCP, 128, DH4).transpose(1, 0, 2)
            )

        m["wq"] = _wblock(Wq)
        m["wk"] = _wblock(Wk)
        m["wv"] = _wblock(Wv)
        bq2 = np.asarray(bq, f32)[cs].reshape(2, 128)
        bk2 = np.asarray(bk, f32)[cs].reshape(2, 128)
        m["bqkv"] = np.ascontiguousarray(
            np.stack([bq2, bk2], axis=1).transpose(2, 1, 0)
        )  # [128, 2(t), 2(m)]
        wos = np.asarray(Wo, f32)[cs, :].astype(bf16)  # [256, D]
        m["wo"] = np.ascontiguousarray(wos.reshape(2, 128, D).transpose(1, 0, 2))
        in_maps.append(m)
    return in_maps


def kernel(query, key_, value, mask, Wq, bq, Wk, bk, Wv, bv, Wo, bo):
    from concourse.bass_utils import run_bass_kernel_spmd

    if "nc" not in _CACHE:
        _CACHE["nc"] = _build_nc()
    nc = _CACHE["nc"]

    in_maps = _prep_inputs(
        query, key_, value, mask, Wq, bq, Wk, bk, Wv, bv, Wo, bo
    )
    res = run_bass_kernel_spmd(nc, in_maps, core_ids=list(range(NCORES))).results

    out = np.zeros((B, S, D), np.float32)
    for c in range(NCORES):
        out[c // (NCORES // B)] += res[c]["out"]
    out += (
        np.asarray(bv, np.float32) @ np.asarray(Wo, np.float32)
        + np.asarray(bo, np.float32)
    )[None, None, :]
    return out
